# revision 28
# baseline (speedup 1.0000x reference)
"""Fused-embedding attention kernel for Trainium2 (8 NeuronCores, SPMD).

Problem (per batch element b, with S=2048, E=256+256+512=1024):
    x = concat(e1, e2, e3)                 # [S, E]
    q/k/v = x @ W{q,k,v}.T + b{q,k,v}      # [S, E]
    probs = softmax(q @ k.T)               # [S, S]
    out = leakyrelu(probs @ v @ Wf.T + bf, 0.2)

Sharding: data-parallel over batch — core c owns batch element c.

Host-side prep is layout + weight algebra (constant folding on weights):
    M    = Wq.T @ Wk          so q @ k.T == x M x.T + (bq@Wk) x.T + const(row)
                              (the row-constant and bk terms cancel in softmax)
    G    = (Wf @ Wv).T        so probs @ v @ Wf.T == probs @ (x G) + r*bvf
    c    = bq @ Wk            (query-side score bias; shipped as a [128,8]
                              column tile and folded into the qMT PSUM->SBUF
                              copy as a per-partition ACT bias)
    bffB = Wf @ bv + bf       (output bias, shipped partition-replicated
                              [128,E] and folded into xG by the DVE copy:
                              expT.T @ (xG + 1*bff) == out_unnorm + r*bff)
The device then runs only the four core GEMM streams per batch element:
    qMT[e',s]  = (x M + 1c)^T  (fp32r; ACT Identity+bias does the +c)
    xG[s,o]    = x G + 1 bff   (fp32r matmuls; DVE/Pool add bff, cast bf16)
    scoresT    = xT.T @ qMT    (fp32r, transposed layout [sk, sq])
    expT       = exp(scoresT)  (ACT, bf16; softmax without max subtraction —
                                scores are bounded ~|65| for this model)
    r          = colsum(expT)  (DVE partial sums over sk-tiles + one K=128
                                ones-matmul per chunk; no per-sk matmuls)
    out[sq,o]  = expT.T @ xG   (bf16 matmuls; bias already inside xG)
    out        = Prelu(out * (1/r), alpha=0.2)  (one ACT pass)
All PSUM comes from a single 8-bank ring so phase boundaries hand off
per-bank instead of draining the whole pool.  The reciprocal runs on the
[P,4] column form (after the PE transposes) — a [1,512] single-partition
reciprocal serializes on one DVE lane and stalls the PE.
"""

import numpy as np

import concourse.bass as bass
import concourse.tile as tile
from concourse import mybir
from concourse.bass_utils import run_bass_kernel_spmd

B, S, E = 8, 2048, 1024
P = 128
ET = E // P            # 8 e-tiles
ST = S // P            # 16 s-tiles
NJ = 4                 # 512-wide s-chunks per xT tile row
SJ = S // NJ           # 512
CHUNK = 512            # sq chunk width in phase 2
NCHUNK = S // CHUNK
NSUB = CHUNK // P      # sq sub-tiles per chunk
HALF = E // 2
NEG_SLOPE = 0.2

F32R = mybir.dt.float32r
F32 = mybir.dt.float32
F16 = mybir.dt.float16
BF16 = mybir.dt.bfloat16
Act = mybir.ActivationFunctionType
Alu = mybir.AluOpType

_CACHE = {}


def _split_multiwait(nc, maxw=1):
    """Split instructions carrying more sync waits than this walrus build can
    encode (one per instruction) into preceding single-wait nops."""
    uid = [0]
    for fn in nc.m.functions:
        for bb in fn.blocks:
            insts = bb.instructions
            if not any(
                i.sync_info is not None
                and i.sync_info.on_wait
                and len(i.sync_info.on_wait) > maxw
                for i in insts
            ):
                continue
            new = []
            for inst in insts:
                si = inst.sync_info
                waits = list(si.on_wait) if si is not None and si.on_wait else []
                if len(waits) > maxw:
                    head, keep = waits[:-maxw], waits[-maxw:]
                    for j in range(0, len(head), maxw):
                        uid[0] += 1
                        new.append(
                            mybir.InstNoOp(
                                name=f"waitsplit-{uid[0]}",
                                sync_info=mybir.SyncInfo(
                                    on_wait=head[j : j + maxw], on_update=[]
                                ),
                                bass_nofuse=True,
                                engine=inst.engine,
                            )
                        )
                    si.on_wait = keep
                new.append(inst)
            bb.instructions = new
    return nc


def _build_nc():
    nc = bass.Bass("TRN2", target_bir_lowering=False, debug=False)

    xT_d = nc.declare_dram_parameter("xT", [E, S], F16, isOutput=False)
    M_d = nc.declare_dram_parameter("M", [E, E], F16, isOutput=False)
    G_d = nc.declare_dram_parameter("G", [E, E], F16, isOutput=False)
    ccol_d = nc.declare_dram_parameter("c_cols", [P, ET], F32, isOutput=False)
    id128_d = nc.declare_dram_parameter("id128", [P, P], F32R, isOutput=False)
    bffB_d = nc.declare_dram_parameter("bffB", [P, E], F32, isOutput=False)
    out_d = nc.declare_dram_parameter("out", [S, E], BF16, isOutput=True)

    with tile.TileContext(nc) as tc:
        _emit(nc, tc, xT_d, M_d, G_d, ccol_d, id128_d, bffB_d, out_d)
    return nc


def _emit(nc, tc, xT_d, M_d, G_d, ccol_d, id128_d, bffB_d, out_d):
    from contextlib import ExitStack

    with ExitStack() as ctx:
        # ---------------- persistent pools (live through both phases)
        consts = ctx.enter_context(tc.tile_pool(name="consts", bufs=1))
        xt_pool = ctx.enter_context(tc.tile_pool(name="xt", bufs=1))
        qm_pool = ctx.enter_context(tc.tile_pool(name="qm", bufs=1))
        xg_pool = ctx.enter_context(tc.tile_pool(name="xg", bufs=1))
        # single 8-bank PSUM ring shared by both phases
        psum = ctx.enter_context(tc.tile_pool(name="psum", bufs=8, space="PSUM"))
        _psn = [0]

        def ps_tile(shape=None):
            _psn[0] += 1
            return psum.tile(shape or [P, CHUNK], F32, tag="ps",
                             name=f"ps_{_psn[0]}")

        ccol_sb = consts.tile([P, ET], F32, tag="ccol")
        nc.gpsimd.dma_start(ccol_sb[:], ccol_d[:])
        id128_sb = consts.tile([P, P], F32R, tag="id128")
        alpha_t = consts.tile([P, 1], F32, tag="alpha")
        nc.vector.memset(alpha_t[:], NEG_SLOPE)
        act_warm = consts.tile([P, 1], F32, tag="act_warm")
        pe_warm = consts.tile([P, SJ], F16, tag="pe_warm")
        nc.vector.memset(pe_warm[:], 0.0)
        bffB_sb = consts.tile([P, E], F32, tag="bffB")

        # residents: xT [p=e%128][et][j] -> 32 tiles [P, 512] so the first
        # projection matmuls start as soon as their slice lands; DMA issue is
        # interleaved w[et] -> xT[et][0..3] in the exact order phase 1
        # consumes them, spread over the three DMA-issue engines.
        xT_r = xT_d.rearrange("(et p) s -> p et s", p=P)
        M_r = M_d.rearrange("(et p) o -> p et o", p=P)
        G_r = G_d.rearrange("(et p) o -> p et o", p=P)

        dma_engs = [nc.sync, nc.gpsimd]
        qMT_sb = qm_pool.tile([P, ET, S], F16)
        xG_sb = xg_pool.tile([P, ST, E], BF16)

        # ---------------- phase 1: qMT = (x M + 1c)^T ; xG = x G + 1 bff
        with tc.tile_pool(name="wstream", bufs=2) as w_pool:
            def load_w(Wr, half, wname, eng):
                ws = []
                for et in range(ET):
                    w = w_pool.tile([P, HALF], F16, tag=f"wet{et}",
                                    name=f"w_{wname}_{half}_{et}")
                    eng.dma_start(w[:], Wr[:, et, bass.ts(half, HALF)])
                    ws.append(w)
                return ws

            # DMA order matches the j-major first sweep: all of w_A plus the
            # j=0 column of xT first (4 MB), then the j=1..3 columns — so the
            # first qMT group can start after ~4 MB instead of 10 MB, and the
            # per-group feed rate (2 MB / group) keeps up afterward.
            w_A = []
            xT_t = [[None] * NJ for _ in range(ET)]
            for et in range(ET):
                w = w_pool.tile([P, HALF], F16, tag=f"wet{et}",
                                name=f"w_M_0_{et}")
                nc.scalar.dma_start(w[:], M_r[:, et, 0:HALF])
                w_A.append(w)
                for j in range(2):
                    t = xt_pool.tile([P, SJ], F16, tag=f"xt{et}_{j}")
                    dma_engs[(et + j) % 2].dma_start(
                        t[:], xT_r[:, et, bass.ts(j, SJ)]
                    )
                    xT_t[et][j] = t
            for j in range(2, NJ):
                for et in range(ET):
                    t = xt_pool.tile([P, SJ], F16, tag=f"xt{et}_{j}")
                    dma_engs[(et + j) % 2].dma_start(
                        t[:], xT_r[:, et, bass.ts(j, SJ)]
                    )
                    xT_t[et][j] = t
            # PE clock warmup: the p-state ramps only while the PE executes,
            # so ~4us of throwaway matmuls during the first-DMA wait bring the
            # clock to full speed before the real first matmul lands.  The
            # result lands in PSUM ring slot 0 and is never read.
            warm_ps = ps_tile()
            for wi in range(12):
                nc.tensor.matmul(
                    warm_ps[0:1, :],
                    pe_warm[:, 0:1],
                    pe_warm[:],
                    start=(wi == 0),
                    stop=(wi == 11),
                )
            # first ACT op triggers a ~1.3us ACT_TABLE_LOAD; fire it right
            # after the w_A descriptors so it lands in the DMA-bound window
            # instead of stalling the first qMT copy (and with it the PSUM
            # ring) at full speed.  All later weight streams are prefetched
            # here but their descriptor generation goes to sync/gpsimd — the
            # scalar sequencer must reach the qMT copies quickly.
            nc.scalar.copy(act_warm[:], alpha_t[:])
            w_B = load_w(M_r, 1, "M", nc.sync)
            nc.sync.dma_start(bffB_sb[:], bffB_d[:])
            w_GA = load_w(G_r, 0, "G", nc.gpsimd)
            nc.gpsimd.dma_start(id128_sb[:], id128_d[:])
            w_GB = load_w(G_r, 1, "G", nc.gpsimd)

            # qMT[e',s] = sum_e M[e,e'].T @ xT[e,s]; +c via ACT Identity bias.
            # Groups are j-major (one 512-wide s-column, all 4 ol of a half)
            # so each group consumes only w + one xT column.
            for half in range(2):
                w_sb = w_A if half == 0 else w_B
                for jg in range(0, NJ, 2):
                    pss = {}
                    for j in (jg, jg + 1):
                        for ol in range(HALF // P):
                            pss[(j, ol)] = ps_tile()
                    for et in range(ET):
                        for j in (jg, jg + 1):
                            for ol in range(HALF // P):
                                nc.tensor.matmul(
                                    pss[(j, ol)][:],
                                    w_sb[et][:, bass.ts(ol, P)],
                                    xT_t[et][j][:],
                                    start=(et == 0),
                                    stop=(et == ET - 1),
                                )
                    for j in (jg, jg + 1):
                        for ol in range(HALF // P):
                            ot = half * (HALF // P) + ol
                            nc.scalar.activation(
                                qMT_sb[:, ot, bass.ts(j, SJ)],
                                pss[(j, ol)][:],
                                Act.Identity,
                                bias=ccol_sb[:, ot : ot + 1],
                            )

            # xG[s,o] = sum_e xT[e,s].T @ G[e,o]; DVE adds bff, casts bf16
            # (gpsimd cannot read PSUM)
            cp_engs = [nc.vector, nc.vector]
            for half in range(2):
                w_sb = w_GA if half == 0 else w_GB
                osl = bass.ts(half, HALF)
                for st0 in range(0, ST, ST // 2):
                    pss = [ps_tile() for _ in range(8)]
                    for et in range(ET):
                        for k in range(8):
                            st = st0 + k
                            nc.tensor.matmul(
                                pss[k][:],
                                xT_t[et][st // NJ][:, bass.ts(st % NJ, P)],
                                w_sb[et][:],
                                start=(et == 0),
                                stop=(et == ET - 1),
                            )
                    for k in range(8):
                        cp_engs[k % 2].scalar_tensor_tensor(
                            xG_sb[:, st0 + k, osl],
                            pss[k][:],
                            1.0,
                            bffB_sb[:, osl],
                            Alu.bypass,
                            Alu.add,
                        )

        # ---------------- phase 2: attention, chunked over sq
        with (
            tc.tile_pool(name="expT", bufs=1) as exp_pool,
            tc.tile_pool(name="rrow", bufs=2) as r_pool,
            tc.tile_pool(name="ostage", bufs=2) as o_pool,
        ):
            for c in range(NCHUNK):
                csl = bass.ds(c * CHUNK, CHUNK)

                expT = exp_pool.tile([P, ST, CHUNK], BF16, tag="e")
                r_part = r_pool.tile([P, CHUNK], F32R, tag=f"rp{c % 2}")

                # --- scoresT + exp; DVE accumulates column sums over sk
                for sk in range(ST):
                    ps = ps_tile()
                    for et in range(ET):
                        nc.tensor.matmul(
                            ps[:],
                            xT_t[et][sk // NJ][:, bass.ts(sk % NJ, P)],
                            qMT_sb[:, et, csl],
                            start=(et == 0),
                            stop=(et == ET - 1),
                        )
                    nc.scalar.activation(expT[:, sk, :], ps[:], Act.Exp)
                    if sk == 1:
                        nc.vector.scalar_tensor_tensor(
                            r_part[:], expT[:, 0, :], 1.0, expT[:, 1, :],
                            Alu.bypass, Alu.add,
                        )
                    elif sk > 1:
                        nc.vector.scalar_tensor_tensor(
                            r_part[:], expT[:, sk, :], 1.0, r_part[:],
                            Alu.bypass, Alu.add,
                        )

                r_col = r_pool.tile([P, NSUB], F32, tag=f"rc{c % 2}")
                rinv_col = r_pool.tile([P, NSUB], F32, tag=f"ri{c % 2}")

                # --- out[sq,o] = expT.T @ xG, normalized + leakyrelu.
                # r machinery: reduce matmul right after m0's matmuls, tiny
                # transposes after m1's, so the PE never waits on the DVE
                # column-sum chain or the ACT row copy.
                ostgs = []
                psss = []
                for m in range(NSUB):
                    ostg = o_pool.tile([P, E], BF16, tag="o")
                    pss = [ps_tile() for _ in range(2)]
                    last_group = c == NCHUNK - 1 and m == NSUB - 1
                    if last_group:
                        # de-interleave the oc halves and emit each prelu+DMA
                        # as soon as its half completes, so the final ACT and
                        # DMA overlap the second half's matmuls instead of
                        # serializing after the very last matmul
                        for pm in range(2):
                            osl = bass.ts(pm, HALF)
                            nc.scalar.activation(
                                ostgs[m - 1][:, osl],
                                psss[m - 1][pm][:],
                                Act.Prelu,
                                scale=rinv_col[:, m - 1 : m],
                                alpha=alpha_t[:],
                            )
                            dma_engs[pm].dma_start(
                                out_d[bass.ds(c * CHUNK + (m - 1) * P, P), osl],
                                ostgs[m - 1][:, osl],
                            )
                        for oc in range(2):
                            osl = bass.ts(oc, HALF)
                            for sk in range(ST):
                                nc.tensor.matmul(
                                    pss[oc][:],
                                    expT[:, sk, bass.ts(m, P)],
                                    xG_sb[:, sk, osl],
                                    start=(sk == 0),
                                    stop=(sk == ST - 1),
                                )
                            nc.scalar.activation(
                                ostg[:, osl],
                                pss[oc][:],
                                Act.Prelu,
                                scale=rinv_col[:, m : m + 1],
                                alpha=alpha_t[:],
                            )
                            dma_engs[oc].dma_start(
                                out_d[bass.ds(c * CHUNK + m * P, P), osl],
                                ostg[:, osl],
                            )
                        continue
                    for sk in range(ST):
                        for oc in range(2):
                            nc.tensor.matmul(
                                pss[oc][:],
                                expT[:, sk, bass.ts(m, P)],
                                xG_sb[:, sk, bass.ts(oc, HALF)],
                                start=(sk == 0),
                                stop=(sk == ST - 1),
                            )
                    ostgs.append(ostg)
                    psss.append(pss)
                    if m == 1:
                        # transpose 128x128 blocks of r_part, then reduce the
                        # folded-sk dim on DVE -> per-partition column sums
                        for j in range(NSUB):
                            rt = psum.tile([P, P], F32R, tag="ps",
                                           name=f"rt_{c}_{j}")
                            nc.tensor.transpose(
                                rt[:], r_part[:, bass.ts(j, P)], id128_sb[:]
                            )
                            nc.vector.reduce_sum(
                                r_col[:, j : j + 1],
                                rt[:].bitcast(F32),
                                axis=mybir.AxisListType.X,
                            )
                        nc.vector.reciprocal(rinv_col[:], r_col[:])
                    if m >= 1:
                        # emit prelu+dma for m-1 (m=1: after transposes) and
                        # for m itself once the last group is done.  One
                        # full-width DMA per row-block halves the descriptor
                        # and completion-semaphore count.
                        for mm_ in ([m - 1] if m < NSUB - 1 else [m - 1, m]):
                            for oc in range(2):
                                osl = bass.ts(oc, HALF)
                                nc.scalar.activation(
                                    ostgs[mm_][:, osl],
                                    psss[mm_][oc][:],
                                    Act.Prelu,
                                    scale=rinv_col[:, mm_ : mm_ + 1],
                                    alpha=alpha_t[:],
                                )
                            dma_engs[mm_ % 2].dma_start(
                                out_d[bass.ds(c * CHUNK + mm_ * P, P), :],
                                ostgs[mm_][:],
                            )


def _host_prep(inputs):
    x = np.concatenate(
        [inputs["embeding1"], inputs["embeding2"], inputs["embeding3"]], axis=-1
    ).astype(np.float32)
    Wq = inputs["Wq"].astype(np.float64)
    Wk = inputs["Wk"].astype(np.float64)
    Wv = inputs["Wv"].astype(np.float64)
    Wf = inputs["Wf"].astype(np.float64)
    bq = inputs["bq"].astype(np.float64)
    bv = inputs["bv"].astype(np.float64)
    bf = inputs["bf"].astype(np.float64)

    c = (bq @ Wk).astype(np.float32)
    bff = (Wf @ bv + bf).astype(np.float32)
    shared = {
        "M": np.ascontiguousarray((Wq.T @ Wk).astype(np.float16)),
        "G": np.ascontiguousarray((Wf @ Wv).T.astype(np.float16)),
        # c in column layout: c_cols[p, ot] = c[ot*128 + p]
        "c_cols": np.ascontiguousarray(c.reshape(ET, P).T),
        "id128": np.eye(P, dtype=np.float32),
        # bff replicated across partitions for the DVE broadcast-add
        "bffB": np.ascontiguousarray(np.tile(bff.reshape(1, E), (P, 1))),
    }
    in_maps = [{"xT": np.ascontiguousarray(x[b].T.astype(np.float16)), **shared} for b in range(B)]
    return in_maps


def kernel(**inputs):
    in_maps = _host_prep(inputs)
    if "nc" not in _CACHE:
        _CACHE["nc"] = _split_multiwait(_build_nc())
    res = run_bass_kernel_spmd(_CACHE["nc"], in_maps, list(range(B)))
    out = np.stack([res.results[b]["out"] for b in range(B)], axis=0)
    return out.astype(np.float32)


if __name__ == "__main__":
    # smoke test in CoreSim on one batch element
    import concourse.bass_interp as bi
    from concourse.bass_interp import CoreSim

    # CoreSim implements neither Lrelu nor Prelu; emulate via Copy + post-fix
    _orig_act = bi.InstructionExecutor.visit_InstActivation

    def _patched_act(self, instruction, *, reg_snapshot=None):
        if instruction.func not in (
            mybir.ActivationFunctionType.Lrelu,
            mybir.ActivationFunctionType.Prelu,
        ):
            return _orig_act(self, instruction, reg_snapshot=reg_snapshot)
        alpha_arg = instruction.ins[3] if len(instruction.ins) > 3 else None
        alpha = (
            alpha_arg.value
            if isinstance(alpha_arg, mybir.ImmediateValue)
            else NEG_SLOPE
        )
        ofunc = instruction.func
        instruction.func = mybir.ActivationFunctionType.Copy
        try:
            _orig_act(self, instruction, reg_snapshot=reg_snapshot)
        finally:
            instruction.func = ofunc
        out_view = self.view_ap(
            instruction.outs[0], bi.Direction.WRITE, instruction,
            reg_snapshot=reg_snapshot,
        )
        out_view[:] = np.where(out_view >= 0, out_view, alpha * out_view)

    bi.InstructionExecutor.visit_InstActivation = _patched_act

    d = np.load("/root/problem/inputs_cache.npz")
    in_maps = _host_prep(dict(d))
    nc = _build_nc()
    sim = CoreSim(nc)
    for k, v in in_maps[0].items():
        sim.tensor(k)[:] = v
    sim.simulate()
    got = np.asarray(sim.tensor("out"))
    ref = np.load("/root/problem/ref_out.npy")[0]
    err = np.abs(got - ref).max() / np.abs(ref).max()
    l2 = np.linalg.norm(got - ref) / np.linalg.norm(ref)
    print(f"SIM scaled absmax err: {err:.3e}  l2 rel: {l2:.3e}")


# revision 29
# speedup vs baseline: 1.0051x; 1.0051x over previous
"""Fused-embedding attention kernel for Trainium2 (8 NeuronCores, SPMD).

Problem (per batch element b, with S=2048, E=256+256+512=1024):
    x = concat(e1, e2, e3)                 # [S, E]
    q/k/v = x @ W{q,k,v}.T + b{q,k,v}      # [S, E]
    probs = softmax(q @ k.T)               # [S, S]
    out = leakyrelu(probs @ v @ Wf.T + bf, 0.2)

Sharding: data-parallel over batch — core c owns batch element c.

Host-side prep is layout + weight algebra (constant folding on weights):
    M    = Wq.T @ Wk          so q @ k.T == x M x.T + (bq@Wk) x.T + const(row)
                              (the row-constant and bk terms cancel in softmax)
    G    = (Wf @ Wv).T        so probs @ v @ Wf.T == probs @ (x G) + r*bvf
    c    = bq @ Wk            (query-side score bias; shipped as a [128,8]
                              column tile and folded into the qMT PSUM->SBUF
                              copy as a per-partition ACT bias)
    bffB = Wf @ bv + bf       (output bias, shipped partition-replicated
                              [128,E] and folded into xG by the DVE copy:
                              expT.T @ (xG + 1*bff) == out_unnorm + r*bff)
The device then runs only the four core GEMM streams per batch element:
    qMT[e',s]  = (x M + 1c)^T  (fp32r; ACT Identity+bias does the +c)
    xG[s,o]    = x G + 1 bff   (fp32r matmuls; DVE/Pool add bff, cast bf16)
    scoresT    = xT.T @ qMT    (fp32r, transposed layout [sk, sq])
    expT       = exp(scoresT)  (ACT, bf16; softmax without max subtraction —
                                scores are bounded ~|65| for this model)
    r          = colsum(expT)  (DVE partial sums over sk-tiles + one K=128
                                ones-matmul per chunk; no per-sk matmuls)
    out[sq,o]  = expT.T @ xG   (bf16 matmuls; bias already inside xG)
    out        = Prelu(out * (1/r), alpha=0.2)  (one ACT pass)
All PSUM comes from a single 8-bank ring so phase boundaries hand off
per-bank instead of draining the whole pool.  The reciprocal runs on the
[P,4] column form (after the PE transposes) — a [1,512] single-partition
reciprocal serializes on one DVE lane and stalls the PE.
"""

import numpy as np

import concourse.bass as bass
import concourse.tile as tile
from concourse import mybir
from concourse.bass_utils import run_bass_kernel_spmd

B, S, E = 8, 2048, 1024
P = 128
ET = E // P            # 8 e-tiles
ST = S // P            # 16 s-tiles
NJ = 4                 # 512-wide s-chunks per xT tile row
SJ = S // NJ           # 512
CHUNK = 512            # sq chunk width in phase 2
NCHUNK = S // CHUNK
NSUB = CHUNK // P      # sq sub-tiles per chunk
HALF = E // 2
NEG_SLOPE = 0.2

F32R = mybir.dt.float32r
F32 = mybir.dt.float32
F16 = mybir.dt.float16
BF16 = mybir.dt.bfloat16
Act = mybir.ActivationFunctionType
Alu = mybir.AluOpType

_CACHE = {}


def _split_multiwait(nc, maxw=1):
    """Split instructions carrying more sync waits than this walrus build can
    encode (one per instruction) into preceding single-wait nops."""
    uid = [0]
    for fn in nc.m.functions:
        for bb in fn.blocks:
            insts = bb.instructions
            if not any(
                i.sync_info is not None
                and i.sync_info.on_wait
                and len(i.sync_info.on_wait) > maxw
                for i in insts
            ):
                continue
            new = []
            for inst in insts:
                si = inst.sync_info
                waits = list(si.on_wait) if si is not None and si.on_wait else []
                if len(waits) > maxw:
                    head, keep = waits[:-maxw], waits[-maxw:]
                    for j in range(0, len(head), maxw):
                        uid[0] += 1
                        new.append(
                            mybir.InstNoOp(
                                name=f"waitsplit-{uid[0]}",
                                sync_info=mybir.SyncInfo(
                                    on_wait=head[j : j + maxw], on_update=[]
                                ),
                                bass_nofuse=True,
                                engine=inst.engine,
                            )
                        )
                    si.on_wait = keep
                new.append(inst)
            bb.instructions = new
    return nc


def _build_nc():
    nc = bass.Bass("TRN2", target_bir_lowering=False, debug=False)

    xT_d = nc.declare_dram_parameter("xT", [E, S], F16, isOutput=False)
    M_d = nc.declare_dram_parameter("M", [E, E], F16, isOutput=False)
    G_d = nc.declare_dram_parameter("G", [E, E], F16, isOutput=False)
    ccol_d = nc.declare_dram_parameter("c_cols", [P, ET], F32, isOutput=False)
    id128_d = nc.declare_dram_parameter("id128", [P, P], F32R, isOutput=False)
    bffB_d = nc.declare_dram_parameter("bffB", [P, E], F32, isOutput=False)
    out_d = nc.declare_dram_parameter("out", [S, E], BF16, isOutput=True)

    with tile.TileContext(nc) as tc:
        _emit(nc, tc, xT_d, M_d, G_d, ccol_d, id128_d, bffB_d, out_d)
    return nc


def _emit(nc, tc, xT_d, M_d, G_d, ccol_d, id128_d, bffB_d, out_d):
    from contextlib import ExitStack

    with ExitStack() as ctx:
        # ---------------- persistent pools (live through both phases)
        consts = ctx.enter_context(tc.tile_pool(name="consts", bufs=1))
        xt_pool = ctx.enter_context(tc.tile_pool(name="xt", bufs=1))
        qm_pool = ctx.enter_context(tc.tile_pool(name="qm", bufs=1))
        xg_pool = ctx.enter_context(tc.tile_pool(name="xg", bufs=1))
        # single 8-bank PSUM ring shared by both phases
        psum = ctx.enter_context(tc.tile_pool(name="psum", bufs=8, space="PSUM"))
        _psn = [0]

        def ps_tile(shape=None):
            _psn[0] += 1
            return psum.tile(shape or [P, CHUNK], F32, tag="ps",
                             name=f"ps_{_psn[0]}")

        ccol_sb = consts.tile([P, ET], F32, tag="ccol")
        nc.gpsimd.dma_start(ccol_sb[:], ccol_d[:])
        id128_sb = consts.tile([P, P], F32R, tag="id128")
        alpha_t = consts.tile([P, 1], F32, tag="alpha")
        nc.vector.memset(alpha_t[:], NEG_SLOPE)
        act_warm = consts.tile([P, 1], F32, tag="act_warm")
        pe_warm = consts.tile([P, SJ], F16, tag="pe_warm")
        nc.vector.memset(pe_warm[:], 0.0)
        bffB_sb = consts.tile([P, E], F32, tag="bffB")

        # residents: xT [p=e%128][et][j] -> 32 tiles [P, 512] so the first
        # projection matmuls start as soon as their slice lands; DMA issue is
        # interleaved w[et] -> xT[et][0..3] in the exact order phase 1
        # consumes them, spread over the three DMA-issue engines.
        xT_r = xT_d.rearrange("(et p) s -> p et s", p=P)
        M_r = M_d.rearrange("(et p) o -> p et o", p=P)
        G_r = G_d.rearrange("(et p) o -> p et o", p=P)

        dma_engs = [nc.sync, nc.gpsimd]
        qMT_sb = qm_pool.tile([P, ET, S], F16)
        xG_sb = xg_pool.tile([P, ST, E], BF16)

        # ---------------- phase 1: qMT = (x M + 1c)^T ; xG = x G + 1 bff
        with tc.tile_pool(name="wstream", bufs=2) as w_pool:
            def load_w(Wr, half, wname, eng):
                ws = []
                for et in range(ET):
                    w = w_pool.tile([P, HALF], F16, tag=f"wet{et}",
                                    name=f"w_{wname}_{half}_{et}")
                    eng.dma_start(w[:], Wr[:, et, bass.ts(half, HALF)])
                    ws.append(w)
                return ws

            # DMA order matches the j-major first sweep: all of w_A plus the
            # j=0 column of xT first (4 MB), then the j=1..3 columns — so the
            # first qMT group can start after ~4 MB instead of 10 MB, and the
            # per-group feed rate (2 MB / group) keeps up afterward.
            w_A = []
            xT_t = [[None] * NJ for _ in range(ET)]
            for et in range(ET):
                w = w_pool.tile([P, HALF], F16, tag=f"wet{et}",
                                name=f"w_M_0_{et}")
                nc.scalar.dma_start(w[:], M_r[:, et, 0:HALF])
                w_A.append(w)
                for j in range(2):
                    t = xt_pool.tile([P, SJ], F16, tag=f"xt{et}_{j}")
                    dma_engs[(et + j) % 2].dma_start(
                        t[:], xT_r[:, et, bass.ts(j, SJ)]
                    )
                    xT_t[et][j] = t
            for j in range(2, NJ):
                for et in range(ET):
                    t = xt_pool.tile([P, SJ], F16, tag=f"xt{et}_{j}")
                    dma_engs[(et + j) % 2].dma_start(
                        t[:], xT_r[:, et, bass.ts(j, SJ)]
                    )
                    xT_t[et][j] = t
            # PE clock warmup: the p-state ramps only while the PE executes,
            # so ~4us of throwaway matmuls during the first-DMA wait bring the
            # clock to full speed before the real first matmul lands.  The
            # result lands in PSUM ring slot 0 and is never read.
            warm_ps = ps_tile()
            for wi in range(12):
                nc.tensor.matmul(
                    warm_ps[0:1, :],
                    pe_warm[:, 0:1],
                    pe_warm[:],
                    start=(wi == 0),
                    stop=(wi == 11),
                )
            # first ACT op triggers a ~1.3us ACT_TABLE_LOAD; fire it right
            # after the w_A descriptors so it lands in the DMA-bound window
            # instead of stalling the first qMT copy (and with it the PSUM
            # ring) at full speed.  All later weight streams are prefetched
            # here but their descriptor generation goes to sync/gpsimd — the
            # scalar sequencer must reach the qMT copies quickly.
            nc.scalar.copy(act_warm[:], alpha_t[:])
            w_B = load_w(M_r, 1, "M", nc.sync)
            nc.sync.dma_start(bffB_sb[:], bffB_d[:])
            w_GA = load_w(G_r, 0, "G", nc.gpsimd)
            nc.gpsimd.dma_start(id128_sb[:], id128_d[:])
            w_GB = load_w(G_r, 1, "G", nc.gpsimd)

            # qMT[e',s] = sum_e M[e,e'].T @ xT[e,s]; +c via ACT Identity bias.
            # Groups are j-major (one 512-wide s-column, all 4 ol of a half)
            # so each group consumes only w + one xT column.
            for half in range(2):
                w_sb = w_A if half == 0 else w_B
                for jg in range(0, NJ, 2):
                    pss = {}
                    for j in (jg, jg + 1):
                        for ol in range(HALF // P):
                            pss[(j, ol)] = ps_tile()
                    for et in range(ET):
                        for j in (jg, jg + 1):
                            for ol in range(HALF // P):
                                nc.tensor.matmul(
                                    pss[(j, ol)][:],
                                    w_sb[et][:, bass.ts(ol, P)],
                                    xT_t[et][j][:],
                                    start=(et == 0),
                                    stop=(et == ET - 1),
                                )
                    for j in (jg, jg + 1):
                        for ol in range(HALF // P):
                            ot = half * (HALF // P) + ol
                            nc.scalar.activation(
                                qMT_sb[:, ot, bass.ts(j, SJ)],
                                pss[(j, ol)][:],
                                Act.Identity,
                                bias=ccol_sb[:, ot : ot + 1],
                            )

            # xG[s,o] = sum_e xT[e,s].T @ G[e,o]; DVE adds bff, casts bf16
            # (gpsimd cannot read PSUM)
            cp_engs = [nc.vector, nc.vector]
            for half in range(2):
                w_sb = w_GA if half == 0 else w_GB
                osl = bass.ts(half, HALF)
                for st0 in range(0, ST, ST // 2):
                    pss = [ps_tile() for _ in range(8)]
                    for et in range(ET):
                        for k in range(8):
                            st = st0 + k
                            nc.tensor.matmul(
                                pss[k][:],
                                xT_t[et][st // NJ][:, bass.ts(st % NJ, P)],
                                w_sb[et][:],
                                start=(et == 0),
                                stop=(et == ET - 1),
                            )
                    for k in range(8):
                        cp_engs[k % 2].scalar_tensor_tensor(
                            xG_sb[:, st0 + k, osl],
                            pss[k][:],
                            1.0,
                            bffB_sb[:, osl],
                            Alu.bypass,
                            Alu.add,
                        )

        # ---------------- phase 2: attention, chunked over sq
        with (
            tc.tile_pool(name="expT", bufs=1) as exp_pool,
            tc.tile_pool(name="rrow", bufs=2) as r_pool,
            tc.tile_pool(name="ostage", bufs=2) as o_pool,
        ):
            for c in range(NCHUNK):
                csl = bass.ds(c * CHUNK, CHUNK)

                expT = exp_pool.tile([P, ST, CHUNK], BF16, tag="e")
                r_part = r_pool.tile([P, CHUNK], F32R, tag=f"rp{c % 2}")

                # --- scoresT + exp; DVE accumulates column sums over sk
                for sk in range(ST):
                    ps = ps_tile()
                    for et in range(ET):
                        nc.tensor.matmul(
                            ps[:],
                            xT_t[et][sk // NJ][:, bass.ts(sk % NJ, P)],
                            qMT_sb[:, et, csl],
                            start=(et == 0),
                            stop=(et == ET - 1),
                        )
                    nc.scalar.activation(expT[:, sk, :], ps[:], Act.Exp)
                    if sk == 1:
                        nc.vector.scalar_tensor_tensor(
                            r_part[:], expT[:, 0, :], 1.0, expT[:, 1, :],
                            Alu.bypass, Alu.add,
                        )
                    elif sk > 1:
                        nc.vector.scalar_tensor_tensor(
                            r_part[:], expT[:, sk, :], 1.0, r_part[:],
                            Alu.bypass, Alu.add,
                        )

                r_col = r_pool.tile([P, NSUB], F32, tag=f"rc{c % 2}")
                rinv_col = r_pool.tile([P, NSUB], F32, tag=f"ri{c % 2}")

                # --- out[sq,o] = expT.T @ xG, normalized + leakyrelu.
                # r machinery: reduce matmul right after m0's matmuls, tiny
                # transposes after m1's, so the PE never waits on the DVE
                # column-sum chain or the ACT row copy.
                ostgs = []
                psss = []
                for m in range(NSUB):
                    ostg = o_pool.tile([P, E], BF16, tag="o")
                    pss = [ps_tile() for _ in range(2)]
                    last_group = c == NCHUNK - 1 and m == NSUB - 1
                    if last_group:
                        # de-interleave the oc halves and emit each prelu+DMA
                        # as soon as its half completes, so the final ACT and
                        # DMA overlap the second half's matmuls instead of
                        # serializing after the very last matmul
                        for pm in range(2):
                            osl = bass.ts(pm, HALF)
                            nc.scalar.activation(
                                ostgs[m - 1][:, osl],
                                psss[m - 1][pm][:],
                                Act.Prelu,
                                scale=rinv_col[:, m - 1 : m],
                                alpha=alpha_t[:],
                            )
                            nc.sync.dma_start(
                                out_d[bass.ds(c * CHUNK + (m - 1) * P, P), osl],
                                ostgs[m - 1][:, osl],
                            )
                        for oc in range(2):
                            for sk in range(ST):
                                nc.tensor.matmul(
                                    pss[oc][:],
                                    expT[:, sk, bass.ts(m, P)],
                                    xG_sb[:, sk, bass.ts(oc, HALF)],
                                    start=(sk == 0),
                                    stop=(sk == ST - 1),
                                )
                            # the very last row-block goes out in quarter
                            # pieces so its first DMA starts one ACT pass
                            # earlier
                            for q in range(2):
                                osl = bass.ds(oc * HALF + q * (HALF // 2),
                                              HALF // 2)
                                nc.scalar.activation(
                                    ostg[:, osl],
                                    pss[oc][:, bass.ts(q, HALF // 2)],
                                    Act.Prelu,
                                    scale=rinv_col[:, m : m + 1],
                                    alpha=alpha_t[:],
                                )
                                nc.sync.dma_start(
                                    out_d[bass.ds(c * CHUNK + m * P, P), osl],
                                    ostg[:, osl],
                                )
                        continue
                    for sk in range(ST):
                        for oc in range(2):
                            nc.tensor.matmul(
                                pss[oc][:],
                                expT[:, sk, bass.ts(m, P)],
                                xG_sb[:, sk, bass.ts(oc, HALF)],
                                start=(sk == 0),
                                stop=(sk == ST - 1),
                            )
                    ostgs.append(ostg)
                    psss.append(pss)
                    if m == 1:
                        # transpose 128x128 blocks of r_part, then reduce the
                        # folded-sk dim on DVE -> per-partition column sums
                        for j in range(NSUB):
                            rt = psum.tile([P, P], F32R, tag="ps",
                                           name=f"rt_{c}_{j}")
                            nc.tensor.transpose(
                                rt[:], r_part[:, bass.ts(j, P)], id128_sb[:]
                            )
                            nc.vector.reduce_sum(
                                r_col[:, j : j + 1],
                                rt[:].bitcast(F32),
                                axis=mybir.AxisListType.X,
                            )
                        nc.vector.reciprocal(rinv_col[:], r_col[:])
                    if m >= 1:
                        # emit prelu+dma for m-1 (m=1: after transposes) and
                        # for m itself once the last group is done.  One
                        # full-width DMA per row-block halves the descriptor
                        # and completion-semaphore count.
                        for mm_ in ([m - 1] if m < NSUB - 1 else [m - 1, m]):
                            for oc in range(2):
                                osl = bass.ts(oc, HALF)
                                nc.scalar.activation(
                                    ostgs[mm_][:, osl],
                                    psss[mm_][oc][:],
                                    Act.Prelu,
                                    scale=rinv_col[:, mm_ : mm_ + 1],
                                    alpha=alpha_t[:],
                                )
                            nc.sync.dma_start(
                                out_d[bass.ds(c * CHUNK + mm_ * P, P), :],
                                ostgs[mm_][:],
                            )


def _host_prep(inputs):
    x = np.concatenate(
        [inputs["embeding1"], inputs["embeding2"], inputs["embeding3"]], axis=-1
    ).astype(np.float32)
    Wq = inputs["Wq"].astype(np.float64)
    Wk = inputs["Wk"].astype(np.float64)
    Wv = inputs["Wv"].astype(np.float64)
    Wf = inputs["Wf"].astype(np.float64)
    bq = inputs["bq"].astype(np.float64)
    bv = inputs["bv"].astype(np.float64)
    bf = inputs["bf"].astype(np.float64)

    c = (bq @ Wk).astype(np.float32)
    bff = (Wf @ bv + bf).astype(np.float32)
    shared = {
        "M": np.ascontiguousarray((Wq.T @ Wk).astype(np.float16)),
        "G": np.ascontiguousarray((Wf @ Wv).T.astype(np.float16)),
        # c in column layout: c_cols[p, ot] = c[ot*128 + p]
        "c_cols": np.ascontiguousarray(c.reshape(ET, P).T),
        "id128": np.eye(P, dtype=np.float32),
        # bff replicated across partitions for the DVE broadcast-add
        "bffB": np.ascontiguousarray(np.tile(bff.reshape(1, E), (P, 1))),
    }
    in_maps = [{"xT": np.ascontiguousarray(x[b].T.astype(np.float16)), **shared} for b in range(B)]
    return in_maps


def kernel(**inputs):
    in_maps = _host_prep(inputs)
    if "nc" not in _CACHE:
        _CACHE["nc"] = _split_multiwait(_build_nc())
    res = run_bass_kernel_spmd(_CACHE["nc"], in_maps, list(range(B)))
    out = np.stack([res.results[b]["out"] for b in range(B)], axis=0)
    return out.astype(np.float32)


if __name__ == "__main__":
    # smoke test in CoreSim on one batch element
    import concourse.bass_interp as bi
    from concourse.bass_interp import CoreSim

    # CoreSim implements neither Lrelu nor Prelu; emulate via Copy + post-fix
    _orig_act = bi.InstructionExecutor.visit_InstActivation

    def _patched_act(self, instruction, *, reg_snapshot=None):
        if instruction.func not in (
            mybir.ActivationFunctionType.Lrelu,
            mybir.ActivationFunctionType.Prelu,
        ):
            return _orig_act(self, instruction, reg_snapshot=reg_snapshot)
        alpha_arg = instruction.ins[3] if len(instruction.ins) > 3 else None
        alpha = (
            alpha_arg.value
            if isinstance(alpha_arg, mybir.ImmediateValue)
            else NEG_SLOPE
        )
        ofunc = instruction.func
        instruction.func = mybir.ActivationFunctionType.Copy
        try:
            _orig_act(self, instruction, reg_snapshot=reg_snapshot)
        finally:
            instruction.func = ofunc
        out_view = self.view_ap(
            instruction.outs[0], bi.Direction.WRITE, instruction,
            reg_snapshot=reg_snapshot,
        )
        out_view[:] = np.where(out_view >= 0, out_view, alpha * out_view)

    bi.InstructionExecutor.visit_InstActivation = _patched_act

    d = np.load("/root/problem/inputs_cache.npz")
    in_maps = _host_prep(dict(d))
    nc = _build_nc()
    sim = CoreSim(nc)
    for k, v in in_maps[0].items():
        sim.tensor(k)[:] = v
    sim.simulate()
    got = np.asarray(sim.tensor("out"))
    ref = np.load("/root/problem/ref_out.npy")[0]
    err = np.abs(got - ref).max() / np.abs(ref).max()
    l2 = np.linalg.norm(got - ref) / np.linalg.norm(ref)
    print(f"SIM scaled absmax err: {err:.3e}  l2 rel: {l2:.3e}")
